# revision 1
# baseline (speedup 1.0000x reference)
"""MemoryGate kernel for Trainium2 (8 NeuronCores, SPMD).

Math (per batch b):
    mp   = memory[b] @ W_mem.T                      [M, D]
    S    = hidden[b] @ mp.T / sqrt(D)               [N, M]
    A    = softmax(S, axis=-1)
    ctx  = A @ mp                                   [N, D]
    gate = sigmoid(hidden @ Wg_h.T + ctx @ Wg_c.T + b_gate)
    out  = rmsnorm(hidden + gate * ctx) * norm_w

Sharding: 8 cores = 4 batches x 2 N-halves. Each core computes mp for its
batch (duplicated across the pair) and processes N/2 = 2048 rows.

All matmul operands are bf16 (PE full rate, FWL weight loads, half the
DMA/SBUF traffic of fp32). PSUM accumulates fp32; softmax stats, gate
output, the fused residual path and rmsnorm stay fp32.

Layout strategy (per core, all [partition, free]):
    hiddenT [D, BN]  (host pre-transposed)  -> lhsT for scores and gate-G1
    memT    [E, M],  WmT [E, D]             -> stage A operands
    mpT     [D, M] (scaled by 1/sqrt(D)), mp [M, D] -> DRAM scratch
    scores computed as [n-part, m-free]; softmax along free dim (no max
    subtraction needed: scores ~ N(0,1), exp is safe in fp32)
    attn transposed on PE (128x128 blocks) -> attnT lhsT for ctx
    ctx transposed on PE -> ctxT lhsT for gate-G2
    b_gate added via a K=1 matmul into the same PSUM accumulation
    rmsnorm along free dim in [n-part, d-free] layout
"""

import math
import os
import sys

for _p in ("/opt/trn_rl_repo", "/root/.axon_site/_ro/trn_rl_repo"):
    if os.path.isdir(_p) and _p not in sys.path:
        sys.path.append(_p)

import numpy as np

P = 128


def build_program(BN=2048, M=2048, D=2048, E=1024, NB=512, FC=512,
                  stop_after=None, b_repeat=1):
    """Build the per-core Bass program. All shapes must divide evenly.

    stop_after: debug aid — truncate the program after a named phase
    ("A", "scores", "attnT", "ctx", "ctxT", "gate"); None = full kernel.
    """
    import concourse.tile as tile
    from concourse import bacc, mybir

    f32 = mybir.dt.float32
    bf16 = mybir.dt.bfloat16
    AF = mybir.ActivationFunctionType
    ALU = mybir.AluOpType
    AX = mybir.AxisListType

    kE, kD, mT, nT = E // P, D // P, M // P, NB // P
    NBLK = BN // NB
    mFC, dFC = M // FC, D // FC
    SCALE = 1.0 / math.sqrt(D)
    EPS = 1e-6

    nc = bacc.Bacc("TRN2", target_bir_lowering=False, debug=False)

    hT = nc.dram_tensor("hiddenT", [D, BN], bf16, kind="ExternalInput")
    hid = nc.dram_tensor("hidden", [BN, D], f32, kind="ExternalInput")
    memT = nc.dram_tensor("memT", [E, M], bf16, kind="ExternalInput")
    WmT = nc.dram_tensor("WmT", [E, D], bf16, kind="ExternalInput")
    WghT = nc.dram_tensor("WghT", [D, D], bf16, kind="ExternalInput")
    WgcT = nc.dram_tensor("WgcT", [D, D], bf16, kind="ExternalInput")
    bg = nc.dram_tensor("b_gate", [1, D], bf16, kind="ExternalInput")
    nw = nc.dram_tensor("norm_w", [1, D], f32, kind="ExternalInput")
    idd = nc.dram_tensor("ident", [P, P], bf16, kind="ExternalInput")
    oned = nc.dram_tensor("ones", [1, P], bf16, kind="ExternalInput")
    out = nc.dram_tensor("out", [BN, D], f32, kind="ExternalOutput")

    with tile.TileContext(nc) as tc:
        with (
            tc.tile_pool(name="dram", bufs=1, space="DRAM") as dpool,
            tc.tile_pool(name="const", bufs=1) as const,
        ):
            mp_d = dpool.tile([M, D], bf16, tag="mp", name="mp_d")

            ident = const.tile([P, P], bf16, tag="ident", name="ident_sb")
            nc.sync.dma_start(ident, idd[:])
            ones_sb = const.tile([1, P], bf16, tag="ones", name="ones_sb")
            nc.sync.dma_start(ones_sb, oned[:])
            bias_sb = const.tile([1, D], bf16, tag="bias", name="bias_sb")
            nc.sync.dma_start(bias_sb, bg[:])
            nw_sb = const.tile([P, D], f32, tag="nw", name="nw_sb")
            nc.gpsimd.dma_start(nw_sb, nw[:].partition_broadcast(P))
            eps_t = const.tile([P, 1], f32, tag="eps", name="eps_sb")
            nc.vector.memset(eps_t, EPS)

            # mpT stays resident in SBUF for the whole kernel (16 MiB bf16)
            hold_cm = tc.tile_pool(name="hold", bufs=1)
            hold = hold_cm.__enter__()
            mpT_sb = hold.tile([P, kD, M], bf16, tag="mpT", name="mpT_sb")

            # ---------------- Stage A: mpT (scaled, SBUF) and mp -> DRAM ----
            with (
                tc.tile_pool(name="a_in", bufs=1) as a_in,
                tc.tile_pool(name="a_st", bufs=4) as a_st,
                tc.tile_pool(name="a_ps", bufs=4, space="PSUM") as a_ps,
            ):
                memT_sb = a_in.tile([P, kE, M], bf16, tag="memT", name="memT_sb")
                WmT_sb = a_in.tile([P, kE, D], bf16, tag="WmT", name="WmT_sb")
                for k in range(kE):
                    nc.sync.dma_start(memT_sb[:, k, :], memT[k * P:(k + 1) * P, :])
                    nc.sync.dma_start(WmT_sb[:, k, :], WmT[k * P:(k + 1) * P, :])
                # A1: mpT[d, m] = sum_e WmT[e, d] * memT[e, m], scaled
                for dp in range(kD):
                    for mc in range(mFC):
                        ps = a_ps.tile([P, FC], f32, tag="ps", name=f"a1ps{dp}_{mc}")
                        for k in range(kE):
                            nc.tensor.matmul(
                                ps,
                                WmT_sb[:, k, dp * P:(dp + 1) * P],
                                memT_sb[:, k, mc * FC:(mc + 1) * FC],
                                start=(k == 0), stop=(k == kE - 1),
                            )
                        nc.scalar.mul(
                            mpT_sb[:, dp, mc * FC:(mc + 1) * FC], ps, SCALE)
                # A2: mp[m, d] = sum_e memT[e, m] * WmT[e, d]
                for mp_ in range(mT):
                    for dc in range(dFC):
                        ps = a_ps.tile([P, FC], f32, tag="ps", name=f"a2ps{mp_}_{dc}")
                        for k in range(kE):
                            nc.tensor.matmul(
                                ps,
                                memT_sb[:, k, mp_ * P:(mp_ + 1) * P],
                                WmT_sb[:, k, dc * FC:(dc + 1) * FC],
                                start=(k == 0), stop=(k == kE - 1),
                            )
                        st = a_st.tile([P, FC], bf16, tag="st", name=f"a2st{mp_}_{dc}")
                        nc.scalar.copy(st, ps)
                        nc.sync.dma_start(
                            mp_d[mp_ * P:(mp_ + 1) * P, dc * FC:(dc + 1) * FC], st)

            # ---------------- Stage B: per N-block pipeline -----------------
            with (
                tc.tile_pool(name="b_big", bufs=1) as bb,
                tc.tile_pool(name="b_strm", bufs=6) as strm,
                tc.tile_pool(name="b_sm", bufs=2) as sm,
                tc.tile_pool(name="b_ps", bufs=6, space="PSUM") as bps,
            ):
                for rep_blk in range(b_repeat * NBLK):
                    blk = rep_blk % NBLK
                    n0 = blk * NB
                    hT_sb = bb.tile([P, kD, NB], bf16, tag="hT", name=f"hT{rep_blk}")
                    for k in range(kD):
                        nc.sync.dma_start(hT_sb[:, k, :], hT[k * P:(k + 1) * P, n0:n0 + NB])

                    if stop_after == "A":
                        continue
                    # scores + exp (+ row-chunk sums)
                    attn = bb.tile([P, nT, M], bf16, tag="attn", name=f"attn{rep_blk}")
                    sums = sm.tile([P, nT * mFC], f32, tag="sums", name=f"sums{rep_blk}")
                    rs = sm.tile([P, nT], f32, tag="rs", name=f"rs{rep_blk}")
                    for mc in range(mFC):
                        pss = [bps.tile([P, FC], f32, tag="ps", name=f"sc{rep_blk}_{mc}_{i}")
                               for i in range(nT)]
                        for k in range(kD):
                            for i in range(nT):
                                nc.tensor.matmul(
                                    pss[i], hT_sb[:, k, i * P:(i + 1) * P],
                                    mpT_sb[:, k, mc * FC:(mc + 1) * FC],
                                    start=(k == 0), stop=(k == kD - 1))
                        for i in range(nT):
                            nc.scalar.activation(
                                attn[:, i, mc * FC:(mc + 1) * FC], pss[i], AF.Exp,
                                accum_out=sums[:, i * mFC + mc: i * mFC + mc + 1])
                    # softmax denominators; normalize attn in place
                    for i in range(nT):
                        nc.vector.reduce_sum(
                            out=rs[:, i:i + 1], in_=sums[:, i * mFC:(i + 1) * mFC], axis=AX.X)
                    nc.vector.reciprocal(rs, rs)
                    for i in range(nT):
                        nc.scalar.mul(attn[:, i, :], attn[:, i, :], rs[:, i:i + 1])

                    if stop_after == "scores":
                        continue
                    # transpose attn -> attnT
                    attnT = bb.tile([P, mT, NB], bf16, tag="attnT", name=f"attnT{rep_blk}")
                    for mt in range(mT):
                        tp = bps.tile([P, NB], bf16, tag="ps", name=f"tpa{rep_blk}_{mt}")
                        for i in range(nT):
                            nc.tensor.transpose(
                                tp[:, i * P:(i + 1) * P], attn[:, i, mt * P:(mt + 1) * P], ident)
                        nc.vector.tensor_copy(attnT[:, mt, :], tp)

                    if stop_after == "attnT":
                        continue
                    # ctx = attn @ mp
                    ctxt = bb.tile([P, nT, D], bf16, tag="ctx", name=f"ctx{rep_blk}")
                    for dc in range(dFC):
                        pss = [bps.tile([P, FC], f32, tag="ps", name=f"cx{rep_blk}_{dc}_{i}")
                               for i in range(nT)]
                        for mt in range(mT):
                            ch = strm.tile([P, FC], bf16, tag="rhs", name=f"c_ch{rep_blk}_{dc}_{mt}")
                            nc.sync.dma_start(ch, mp_d[mt * P:(mt + 1) * P, dc * FC:(dc + 1) * FC])
                            for i in range(nT):
                                nc.tensor.matmul(
                                    pss[i], attnT[:, mt, i * P:(i + 1) * P], ch,
                                    start=(mt == 0), stop=(mt == mT - 1))
                        for i in range(nT):
                            nc.scalar.copy(ctxt[:, i, dc * FC:(dc + 1) * FC], pss[i])

                    if stop_after == "ctx":
                        continue
                    # transpose ctx -> ctxT (reuses attnT's slot)
                    ctxT = bb.tile([P, kD, NB], bf16, tag="attnT", name=f"ctxT{rep_blk}")
                    for dt_ in range(kD):
                        tp = bps.tile([P, NB], bf16, tag="ps", name=f"tpc{rep_blk}_{dt_}")
                        for i in range(nT):
                            nc.tensor.transpose(
                                tp[:, i * P:(i + 1) * P], ctxt[:, i, dt_ * P:(dt_ + 1) * P], ident)
                        nc.vector.tensor_copy(ctxT[:, dt_, :], tp)

                    if stop_after == "ctxT":
                        continue
                    # gate = sigmoid(hidden @ WghT + ctx @ WgcT + b_gate)
                    gate = bb.tile([P, nT, D], bf16, tag="attn", name=f"gate{rep_blk}")
                    for dc in range(dFC):
                        pss = [bps.tile([P, FC], f32, tag="ps", name=f"gt{rep_blk}_{dc}_{i}")
                               for i in range(nT)]
                        for k in range(kD):
                            ch = strm.tile([P, FC], bf16, tag="rhs", name=f"g1ch{rep_blk}_{dc}_{k}")
                            nc.sync.dma_start(ch, WghT[k * P:(k + 1) * P, dc * FC:(dc + 1) * FC])
                            for i in range(nT):
                                nc.tensor.matmul(
                                    pss[i], hT_sb[:, k, i * P:(i + 1) * P], ch,
                                    start=(k == 0), stop=False)
                        for k in range(kD):
                            ch = strm.tile([P, FC], bf16, tag="rhs", name=f"g2ch{rep_blk}_{dc}_{k}")
                            nc.sync.dma_start(ch, WgcT[k * P:(k + 1) * P, dc * FC:(dc + 1) * FC])
                            for i in range(nT):
                                nc.tensor.matmul(
                                    pss[i], ctxT[:, k, i * P:(i + 1) * P], ch,
                                    start=False, stop=False)
                        for i in range(nT):
                            nc.tensor.matmul(
                                pss[i], ones_sb, bias_sb[:, dc * FC:(dc + 1) * FC],
                                start=False, stop=True)
                        for i in range(nT):
                            nc.scalar.activation(
                                gate[:, i, dc * FC:(dc + 1) * FC], pss[i], AF.Sigmoid)

                    if stop_after == "gate":
                        continue
                    # fused = hidden + gate*ctx; out = rmsnorm(fused) * norm_w
                    for i in range(nT):
                        hid_t = strm.tile([P, D], f32, tag="hid", bufs=2, name=f"hid{rep_blk}_{i}")
                        nc.sync.dma_start(hid_t, hid[n0 + i * P: n0 + (i + 1) * P, :])
                        fo = strm.tile([P, D], f32, tag="fo", bufs=2, name=f"fo{rep_blk}_{i}")
                        nc.vector.tensor_mul(fo, gate[:, i, :], ctxt[:, i, :])
                        nc.vector.tensor_add(fo, fo, hid_t)
                        if stop_after == "fused1":
                            nc.sync.dma_start(out[n0 + i * P: n0 + (i + 1) * P, :], fo)
                            continue
                        sq = strm.tile([P, D], f32, tag="hid", bufs=2, name=f"sq{rep_blk}_{i}")
                        ssq = sm.tile([P, 1], f32, tag="ssq", name=f"ssq{rep_blk}_{i}")
                        nc.scalar.activation(sq, fo, AF.Square, accum_out=ssq)
                        rstd = sm.tile([P, 1], f32, tag="rstd", name=f"rstd{rep_blk}_{i}")
                        nc.scalar.activation(rstd, ssq, AF.Sqrt, bias=eps_t, scale=1.0 / D)
                        nc.vector.reciprocal(rstd, rstd)
                        if stop_after == "fused2":
                            nc.sync.dma_start(out[n0 + i * P: n0 + (i + 1) * P, :], fo)
                            continue
                        nc.scalar.mul(fo, fo, rstd)
                        nc.vector.tensor_mul(fo, fo, nw_sb)
                        nc.sync.dma_start(out[n0 + i * P: n0 + (i + 1) * P, :], fo)

            hold_cm.__exit__(None, None, None)

    nc.compile()
    return nc


_PROG_CACHE = {}


def _get_program(key, **kw):
    if key not in _PROG_CACHE:
        _PROG_CACHE[key] = build_program(**kw)
    return _PROG_CACHE[key]


def kernel(hidden_states, memory, W_mem, W_gate, b_gate, norm_w):
    from concourse.bass_utils import run_bass_kernel_spmd

    B, N, D = hidden_states.shape
    _, M, E = memory.shape
    NC = 8
    H = NC // B                      # N-splits per batch (2)
    BN = N // H                      # rows per core (2048)

    prog = _get_program(("full", BN, M, D, E), BN=BN, M=M, D=D, E=E)

    import ml_dtypes
    f32 = np.float32
    bf16 = ml_dtypes.bfloat16
    WmT = np.ascontiguousarray(W_mem.T).astype(bf16)
    WghT = np.ascontiguousarray(W_gate[:, :D].T).astype(bf16)
    WgcT = np.ascontiguousarray(W_gate[:, D:].T).astype(bf16)
    bg = np.ascontiguousarray(b_gate[None, :]).astype(bf16)
    nw = np.ascontiguousarray(norm_w[None, :], dtype=f32)
    ident = np.eye(P, dtype=f32).astype(bf16)
    ones = np.ones((1, P), dtype=bf16)

    in_maps = []
    for c in range(NC):
        b, h = c // H, c % H
        hs = hidden_states[b, h * BN:(h + 1) * BN, :]
        in_maps.append({
            "hiddenT": np.ascontiguousarray(hs.T).astype(bf16),
            "hidden": np.ascontiguousarray(hs, dtype=f32),
            "memT": np.ascontiguousarray(memory[b].T).astype(bf16),
            "WmT": WmT, "WghT": WghT, "WgcT": WgcT,
            "b_gate": bg, "norm_w": nw, "ident": ident, "ones": ones,
        })

    res = run_bass_kernel_spmd(prog, in_maps, core_ids=list(range(NC)))
    out = np.empty((B, N, D), dtype=f32)
    for c in range(NC):
        b, h = c // H, c % H
        out[b, h * BN:(h + 1) * BN, :] = res.results[c]["out"]
    return out



# revision 69
# speedup vs baseline: 8.5039x; 8.5039x over previous
"""MemoryGate kernel for Trainium2 (8 NeuronCores, SPMD).

Math (per batch b):
    mp   = memory[b] @ W_mem.T                      [M, D]
    S    = hidden[b] @ mp.T / sqrt(D)               [N, M]
    A    = softmax(S, axis=-1)
    ctx  = A @ mp                                   [N, D]
    gate = sigmoid(hidden @ Wg_h.T + ctx @ Wg_c.T + b_gate)
    out  = rmsnorm(hidden + gate * ctx) * norm_w

Sharding: 8 cores = 4 batches x 2 N-halves. Each core computes mp for its
batch (duplicated across the pair) and processes N/2 = 2048 rows.

All large matmuls run in fp8 (e4m3) with perf_mode=DoubleRow: both
operands hold two contraction planes [K=128p, 2, free] so each
instruction contracts K=256 at 0.5 cycles/row. The natural [P, kchunk, X]
tiling is DoubleRow-compatible by slicing [:, 2j:2j+2, :] (pairing plane
o with o at the same partition). PSUM accumulates fp32. The residual,
softmax statistics and rmsnorm stay fp32; attention probabilities are
kept as UNNORMALIZED exp(s/sqrt(D) - 2) in fp8 (bias -2 keeps the max
under the e4m3 ceiling of 240) and the softmax denominator is applied
per-row during the ctx PSUM->SBUF copy.

Stage A: mpT [D, M] and mp [M, D] are both computed directly from
memT/WmT (two fp8 DR matmul passes, no transposes) and stay RESIDENT in
SBUF as fp8 for the whole kernel (32K/partition each).

Per N-block (NB=512, 4 blocks): scores (DR, lhsT=hiddenT8 pairs,
rhs=mpT8) -> exp into bf16 attn + f32 row sums -> PE-transpose attn ->
attnT fp8 -> ctx (DR) normalized on copy -> PE-transpose ctx -> ctxT
fp8 -> gate G1+G2 (DR, weights streamed from host-PREPACKED contiguous
layouts, one [P, kD, FC] chunk per DMA) + b_gate via a K=1 bf16 matmul
-> sigmoid -> fused residual (DVE/GPSIMD) -> rmsnorm -> bf16 out.

PSUM->SBUF copies and per-row scalings alternate between the ACT and
DVE engines; the residual add and norm_w multiply run on GPSIMD (Pool)
to keep ACT/DVE off the critical path.

Host-side prep: hiddenT8 / WghT / WgcT are prepacked so every streamed
DMA reads fully contiguous 8KB-per-partition runs.
"""

import math
import os
import sys

for _p in ("/opt/trn_rl_repo", "/root/.axon_site/_ro/trn_rl_repo"):
    if os.path.isdir(_p) and _p not in sys.path:
        sys.path.append(_p)

import numpy as np

P = 128


def build_program(BN=2048, M=2048, D=2048, E=1024, NB=512, FC=512,
                  stop_after=None, b_repeat=1, timing_mode=False,
                  loop_repeat=0, no_bias=False, sq_act=0, wg_bufs=3):
    """Build the per-core Bass program. All shapes must divide evenly.

    stop_after: debug aid — truncate the program after a named phase
    ("A", "scores", "attnT", "ctx", "ctxT", "gate"); None = full kernel.
    timing_mode: big I/O tensors become internal DRAM scratch (zero-
    initialized on device); external I/O is a tiny seed/out pair so HW
    wall-time is not dominated by host transfers.
    """
    import concourse.tile as tile
    from concourse import bacc, mybir

    f32 = mybir.dt.float32
    bf16 = mybir.dt.bfloat16
    AF = mybir.ActivationFunctionType
    ALU = mybir.AluOpType
    AX = mybir.AxisListType

    kE, kD, mT, nT = E // P, D // P, M // P, NB // P
    DB = 2 if NB <= 512 else 1   # double-buffer depth for block tiles
    NBLK = BN // NB
    mFC, dFC = M // FC, D // FC
    SCALE = 1.0 / math.sqrt(D)
    EPS = 1e-6

    nc = bacc.Bacc("TRN2", target_bir_lowering=False, debug=False)

    fp8 = mybir.dt.float8e4

    if not timing_mode:
        hT8d = nc.dram_tensor("hiddenT8", [(BN // NB) * P, kD * NB], fp8, kind="ExternalInput")
        hid = nc.dram_tensor("hidden", [BN, D], f32, kind="ExternalInput")
        memT = nc.dram_tensor("memT", [E, M], fp8, kind="ExternalInput")
        WmT = nc.dram_tensor("WmT", [E, D], fp8, kind="ExternalInput")
        WghT = nc.dram_tensor("WghT", [(D // FC) * P, kD * FC], fp8, kind="ExternalInput")
        WgcT = nc.dram_tensor("WgcT", [(D // FC) * P, kD * FC], fp8, kind="ExternalInput")
        bg = nc.dram_tensor("b_gate", [1, D], bf16, kind="ExternalInput")
        nw = nc.dram_tensor("norm_w", [1, D], f32, kind="ExternalInput")
        idd = nc.dram_tensor("ident", [P, P], bf16, kind="ExternalInput")
        oned = nc.dram_tensor("ones", [1, P], bf16, kind="ExternalInput")
        out = nc.dram_tensor("out", [BN, D], bf16, kind="ExternalOutput")
    else:
        seed = nc.dram_tensor("seed", [1, P], f32, kind="ExternalInput")
        outt = nc.dram_tensor("outt", [1, P], bf16, kind="ExternalOutput")

    with tile.TileContext(nc) as tc:
        with (
            tc.tile_pool(name="dram", bufs=1, space="DRAM") as dpool,
            tc.tile_pool(name="const", bufs=1) as const,
        ):
            if timing_mode:
                hT8d = dpool.tile([(BN // NB) * P, kD * NB], fp8, tag="t_hT8", name="t_hT8")
                hid = dpool.tile([BN, D], f32, tag="t_hid", name="t_hid")
                memT = dpool.tile([E, M], fp8, tag="t_memT", name="t_memT")
                WmT = dpool.tile([E, D], fp8, tag="t_WmT", name="t_WmT")
                WghT = dpool.tile([(D // FC) * P, kD * FC], fp8, tag="t_Wgh", name="t_Wgh")
                WgcT = dpool.tile([(D // FC) * P, kD * FC], fp8, tag="t_Wgc", name="t_Wgc")
                bg = dpool.tile([1, D], bf16, tag="t_bg", name="t_bg")
                nw = dpool.tile([1, D], f32, tag="t_nw", name="t_nw")
                idd = dpool.tile([P, P], bf16, tag="t_idd", name="t_idd")
                oned = dpool.tile([1, P], bf16, tag="t_oned", name="t_oned")
                out = dpool.tile([BN, D], bf16, tag="t_out", name="t_out")
                with tc.tile_pool(name="init", bufs=1) as ipool:
                    zb = ipool.tile([P, 2 * D], bf16, tag="zb", name="zb")
                    nc.vector.memset(zb, 0.0)
                    zf = ipool.tile([P, D], f32, tag="zf", name="zf")
                    nc.vector.memset(zf, 0.0)
                    z8 = ipool.tile([P, kD * NB], fp8, tag="z8", name="z8")
                    nc.vector.memset(z8, 0.0)

                    for t, rows, cols in ((hT8d, (BN // NB) * P, kD * NB),
                                          (WghT, (D // FC) * P, kD * FC),
                                          (WgcT, (D // FC) * P, kD * FC),
                                          (memT, E, M), (WmT, E, D)):
                        for r in range(0, rows, P):
                            nc.sync.dma_start(t[r:r + P, :], z8[:, :cols])
                    for r in range(0, BN, P):
                        nc.sync.dma_start(hid[r:r + P, :], zf[:, :D])
                    nc.sync.dma_start(bg[:], zb[:1, :D])
                    nc.sync.dma_start(nw[:], zf[:1, :D])
                    sd = ipool.tile([1, P], f32, tag="sd", name="sd")
                    nc.sync.dma_start(sd, seed[:])
                    nc.sync.dma_start(nw[:1, :P], sd)
                    nc.sync.dma_start(idd[:], zb[:, :P])
                    nc.sync.dma_start(oned[:], zb[:1, :P])

            ident = const.tile([P, P], bf16, tag="ident", name="ident_sb")
            nc.sync.dma_start(ident, idd[:])
            nb1 = const.tile([P, 1], f32, tag="nb1", name="nb1_sb")
            nc.vector.memset(nb1, -2.0)
            ones_sb = const.tile([1, P], bf16, tag="ones", name="ones_sb")
            nc.sync.dma_start(ones_sb, oned[:])
            bias_sb = const.tile([1, D], bf16, tag="bias", name="bias_sb")
            nc.sync.dma_start(bias_sb, bg[:])
            nw_sb = const.tile([P, D], f32, tag="nw", name="nw_sb")
            nc.gpsimd.dma_start(nw_sb, nw[:].partition_broadcast(P))
            eps_t = const.tile([P, 1], f32, tag="eps", name="eps_sb")
            nc.vector.memset(eps_t, EPS)

            # mpT8 (fp8) stays resident in SBUF for the whole kernel; the
            # bf16 staging copy lives only through stage A (for the mp
            # transposes)
            hold_cm = tc.tile_pool(name="hold", bufs=1)
            hold = hold_cm.__enter__()
            mpT8_sb = hold.tile([P, kD, M], fp8, tag="mpT8", name="mpT8_sb")
            mp8_sb = hold.tile([P, mT, D], fp8, tag="mp8", name="mp8_sb")

            # ---------------- Stage A: mpT (unscaled, SBUF); mp -> DRAM ------
            # mpT is NOT pre-scaled by 1/sqrt(D); the scale is folded into the
            # exp activation (exp(SCALE*s)) so mp can be derived from mpT by
            # PE transpose instead of a second full matmul pass.
            with (
                tc.tile_pool(name="a_in", bufs=1) as a_in,
                tc.tile_pool(name="a_ps", bufs=6, space="PSUM") as a_ps,
            ):
                memT_sb = a_in.tile([P, kE, M], fp8, tag="memT", name="memT_sb")
                WmT_sb = a_in.tile([P, kE, D], fp8, tag="WmT", name="WmT_sb")
                for k in range(kE):
                    nc.sync.dma_start(memT_sb[:, k, :], memT[k * P:(k + 1) * P, :])
                    nc.sync.dma_start(WmT_sb[:, k, :], WmT[k * P:(k + 1) * P, :])
                # A1: mpT[d, m] = sum_e WmT[e, d] * memT[e, m]  (fp8 DR)
                for dp in range(kD):
                    for mc in range(mFC):
                        ps = a_ps.tile([P, FC], f32, tag="ps", name=f"a1ps{dp}_{mc}")
                        for kj in range(kE // 2):
                            nc.tensor.matmul(
                                ps,
                                WmT_sb[:, 2 * kj:2 * kj + 2, dp * P:(dp + 1) * P],
                                memT_sb[:, 2 * kj:2 * kj + 2, mc * FC:(mc + 1) * FC],
                                start=(kj == 0), stop=(kj == kE // 2 - 1),
                                perf_mode=mybir.MatmulPerfMode.DoubleRow,
                            )
                        if (dp + mc) % 2 == 0:
                            nc.scalar.copy(
                                mpT8_sb[:, dp, mc * FC:(mc + 1) * FC], ps)
                        else:
                            nc.vector.tensor_copy(
                                mpT8_sb[:, dp, mc * FC:(mc + 1) * FC], ps)
                # A2: mp[m, d] = sum_e memT[e, m] * WmT[e, d]  (fp8 DR)
                for mp_ in range(mT):
                    for dc in range(dFC):
                        ps = a_ps.tile([P, FC], f32, tag="ps", name=f"a2ps{mp_}_{dc}")
                        for kj in range(kE // 2):
                            nc.tensor.matmul(
                                ps,
                                memT_sb[:, 2 * kj:2 * kj + 2, mp_ * P:(mp_ + 1) * P],
                                WmT_sb[:, 2 * kj:2 * kj + 2, dc * FC:(dc + 1) * FC],
                                start=(kj == 0), stop=(kj == kE // 2 - 1),
                                perf_mode=mybir.MatmulPerfMode.DoubleRow,
                            )
                        if (mp_ + dc) % 2 == 0:
                            nc.vector.tensor_copy(
                                mp8_sb[:, mp_, dc * FC:(dc + 1) * FC], ps)
                        else:
                            nc.scalar.copy(
                                mp8_sb[:, mp_, dc * FC:(dc + 1) * FC], ps)

            # ---------------- Stage B: per N-block pipeline -----------------
            with (
                tc.tile_pool(name="b_big", bufs=1) as bb,
                tc.tile_pool(name="b_strm", bufs=6) as strm,
                tc.tile_pool(name="b_sm", bufs=2) as sm,
                tc.tile_pool(name="b_ps", bufs=8, space="PSUM") as bps,
            ):
                loop_cm = None
                if loop_repeat:
                    loop_cm = tc.For_i(0, loop_repeat)
                    loop_cm.__enter__()
                for rep_blk in range(b_repeat * NBLK):
                    blk = rep_blk % NBLK
                    n0 = blk * NB
                    hT8_sb = bb.tile([P, kD, NB], fp8, tag="hT8", name=f"hT8_{rep_blk}")
                    nc.sync.dma_start(
                        hT8_sb,
                        hT8d[blk * P:(blk + 1) * P, :]
                        .rearrange("p (k n) -> p k n", k=kD))

                    if stop_after == "A":
                        continue
                    # scores -> raw exp(s/sqrt(D) - 1) in fp8 (unnormalized;
                    # softmax denominator applied at the ctx PSUM->SBUF copy)
                    attn = bb.tile([P, nT, M], bf16, tag="attn", name=f"attn{rep_blk}")
                    sums = sm.tile([P, nT * mFC], f32, tag="sums", name=f"sums{rep_blk}")
                    rs = sm.tile([P, nT], f32, tag="rs", name=f"rs{rep_blk}")
                    for mc in range(mFC):
                        pss = [bps.tile([P, FC], f32, tag="ps", name=f"sc{rep_blk}_{mc}_{i}")
                               for i in range(nT)]
                        for kj in range(kD // 2):
                            for i in range(nT):
                                nc.tensor.matmul(
                                    pss[i],
                                    hT8_sb[:, 2 * kj:2 * kj + 2, i * P:(i + 1) * P],
                                    mpT8_sb[:, 2 * kj:2 * kj + 2,
                                            mc * FC:(mc + 1) * FC],
                                    start=(kj == 0), stop=(kj == kD // 2 - 1),
                                    perf_mode=mybir.MatmulPerfMode.DoubleRow)
                        for i in range(nT):
                            nc.scalar.activation(
                                attn[:, i, mc * FC:(mc + 1) * FC], pss[i], AF.Exp,
                                scale=SCALE, bias=nb1,
                                accum_out=sums[:, i * mFC + mc: i * mFC + mc + 1])
                    # softmax denominators (reciprocal row sums)
                    for i in range(nT):
                        nc.vector.reduce_sum(
                            out=rs[:, i:i + 1], in_=sums[:, i * mFC:(i + 1) * mFC], axis=AX.X)
                    nc.vector.reciprocal(rs, rs)

                    if stop_after == "scores":
                        continue
                    # transpose attn -> attnT (fp8)
                    attnT = bb.tile([P, mT, NB], fp8, tag="attnT8", bufs=2, name=f"attnT{rep_blk}")
                    for mt in range(mT):
                        tp = bps.tile([P, NB], bf16, tag="ps", name=f"tpa{rep_blk}_{mt}")
                        for i in range(nT):
                            nc.tensor.transpose(
                                tp[:, i * P:(i + 1) * P], attn[:, i, mt * P:(mt + 1) * P], ident)
                        if mt % 2 == 0:
                            nc.vector.tensor_copy(attnT[:, mt, :], tp)
                        else:
                            nc.scalar.copy(attnT[:, mt, :], tp)

                    if stop_after == "attnT":
                        continue
                    # ctx = attn @ mp  (fp8 DoubleRow, K=256 per matmul;
                    # normalization folded into the PSUM->SBUF copy)
                    ctxt = bb.tile([P, nT, D], bf16, tag="ctx", name=f"ctx{rep_blk}")
                    hMT = mT // 2
                    for dc in range(dFC):
                        pss = [bps.tile([P, FC], f32, tag="ps", name=f"cx{rep_blk}_{dc}_{i}")
                               for i in range(nT)]
                        for mj in range(mT // 2):
                            for i in range(nT):
                                nc.tensor.matmul(
                                    pss[i],
                                    attnT[:, 2 * mj:2 * mj + 2, i * P:(i + 1) * P],
                                    mp8_sb[:, 2 * mj:2 * mj + 2,
                                           dc * FC:(dc + 1) * FC],
                                    start=(mj == 0), stop=(mj == mT // 2 - 1),
                                    perf_mode=mybir.MatmulPerfMode.DoubleRow)
                        for i in range(nT):
                            if i % 2 == 0:
                                nc.vector.tensor_scalar_mul(
                                    ctxt[:, i, dc * FC:(dc + 1) * FC],
                                    pss[i], rs[:, i:i + 1])
                            else:
                                nc.scalar.mul(
                                    ctxt[:, i, dc * FC:(dc + 1) * FC],
                                    pss[i], rs[:, i:i + 1])

                    if stop_after == "ctx":
                        continue
                    # transpose ctx -> ctxT (fp8, reuses attnT's slot)
                    ctxT = bb.tile([P, kD, NB], fp8, tag="attnT8", bufs=2, name=f"ctxT{rep_blk}")
                    for dt_ in range(kD):
                        tp = bps.tile([P, NB], bf16, tag="ps", name=f"tpc{rep_blk}_{dt_}")
                        for i in range(nT):
                            nc.tensor.transpose(
                                tp[:, i * P:(i + 1) * P], ctxt[:, i, dt_ * P:(dt_ + 1) * P], ident)
                        if dt_ % 2 == 0:
                            nc.vector.tensor_copy(ctxT[:, dt_, :], tp)
                        else:
                            nc.scalar.copy(ctxT[:, dt_, :], tp)

                    if stop_after == "ctxT":
                        continue
                    # gate = sigmoid(hidden @ WghT + ctx @ WgcT + b_gate)
                    gate = bb.tile([P, nT, D], bf16, tag="gate", name=f"gate{rep_blk}")
                    hKD = kD // 2
                    for dc in range(dFC):
                        pss = [bps.tile([P, FC], f32, tag="ps", name=f"gt{rep_blk}_{dc}_{i}")
                               for i in range(nT)]
                        ch = strm.tile([P, kD, FC], fp8, tag="wg", bufs=wg_bufs,
                                       name=f"g1ch{rep_blk}_{dc}")
                        nc.sync.dma_start(
                            ch,
                            WghT[dc * P:(dc + 1) * P, :]
                            .rearrange("p (t f) -> p t f", t=kD))
                        for kj in range(kD // 2):
                            for i in range(nT):
                                nc.tensor.matmul(
                                    pss[i],
                                    hT8_sb[:, 2 * kj:2 * kj + 2, i * P:(i + 1) * P],
                                    ch[:, 2 * kj:2 * kj + 2, :],
                                    start=(kj == 0), stop=False,
                                    perf_mode=mybir.MatmulPerfMode.DoubleRow)
                        ch = strm.tile([P, kD, FC], fp8, tag="wg", bufs=wg_bufs,
                                       name=f"g2ch{rep_blk}_{dc}")
                        nc.sync.dma_start(
                            ch,
                            WgcT[dc * P:(dc + 1) * P, :]
                            .rearrange("p (t f) -> p t f", t=kD))
                        for kj in range(kD // 2):
                            for i in range(nT):
                                nc.tensor.matmul(
                                    pss[i],
                                    ctxT[:, 2 * kj:2 * kj + 2, i * P:(i + 1) * P],
                                    ch[:, 2 * kj:2 * kj + 2, :],
                                    start=False, stop=False,
                                    perf_mode=mybir.MatmulPerfMode.DoubleRow)
                        if not no_bias:
                            for i in range(nT):
                                nc.tensor.matmul(
                                    pss[i], ones_sb,
                                    bias_sb[:, dc * FC:(dc + 1) * FC],
                                    start=False, stop=True)
                        for i in range(nT):
                            nc.scalar.activation(
                                gate[:, i, dc * FC:(dc + 1) * FC], pss[i], AF.Sigmoid)

                    if stop_after == "gate":
                        continue
                    # fused = hidden + gate*ctx; out = rmsnorm(fused) * norm_w
                    for i in range(nT):
                        hid_t = strm.tile([P, D], f32, tag="hid", bufs=2, name=f"hid{rep_blk}_{i}")
                        nc.sync.dma_start(hid_t, hid[n0 + i * P: n0 + (i + 1) * P, :])
                        fo = strm.tile([P, D], f32, tag="fo", bufs=2, name=f"fo{rep_blk}_{i}")
                        nc.vector.tensor_mul(fo, gate[:, i, :], ctxt[:, i, :])
                        nc.gpsimd.tensor_add(fo, fo, hid_t)
                        if stop_after == "fused1":
                            nc.sync.dma_start(out[n0 + i * P: n0 + (i + 1) * P, :], fo)
                            continue
                        # squares land in the dead gate slice; only the f32
                        # accum (row sum of squares) is kept
                        ssq = sm.tile([P, 1], f32, tag="ssq", name=f"ssq{rep_blk}_{i}")
                        if sq_act:
                            nc.scalar.activation(gate[:, i, :], fo, AF.Square,
                                                 accum_out=ssq)
                        else:
                            nc.gpsimd.tensor_mul(gate[:, i, :], fo, fo)
                            nc.vector.reduce_sum(out=ssq, in_=gate[:, i, :],
                                                 axis=AX.X)
                        rstd = sm.tile([P, 1], f32, tag="rstd", name=f"rstd{rep_blk}_{i}")
                        nc.scalar.activation(rstd, ssq, AF.Sqrt, bias=eps_t, scale=1.0 / D)
                        nc.vector.reciprocal(rstd, rstd)
                        if stop_after == "fused2":
                            nc.sync.dma_start(out[n0 + i * P: n0 + (i + 1) * P, :], fo)
                            continue
                        ob = strm.tile([P, D], bf16, tag="hid", bufs=2, name=f"ob{rep_blk}_{i}")
                        nc.vector.tensor_scalar_mul(ob, fo, rstd)
                        nc.gpsimd.tensor_mul(ob, ob, nw_sb)
                        nc.sync.dma_start(out[n0 + i * P: n0 + (i + 1) * P, :], ob)

                if loop_cm is not None:
                    loop_cm.__exit__(None, None, None)

            hold_cm.__exit__(None, None, None)

            if timing_mode:
                with tc.tile_pool(name="fin", bufs=1) as fin:
                    ft = fin.tile([1, P], bf16, tag="ft", name="ft")
                    nc.sync.dma_start(ft, out[BN - 1:BN, :P])
                    nc.sync.dma_start(outt[:], ft)

    nc.compile()
    return nc


_PROG_CACHE = {}


def _get_program(key, **kw):
    if key not in _PROG_CACHE:
        _PROG_CACHE[key] = build_program(**kw)
    return _PROG_CACHE[key]


def kernel(hidden_states, memory, W_mem, W_gate, b_gate, norm_w):
    from concourse.bass_utils import run_bass_kernel_spmd

    B, N, D = hidden_states.shape
    _, M, E = memory.shape
    NC = 8
    H = NC // B                      # N-splits per batch (2)
    BN = N // H                      # rows per core (2048)

    prog = _get_program(("full", BN, M, D, E), BN=BN, M=M, D=D, E=E)

    import ml_dtypes
    f32 = np.float32
    bf16 = ml_dtypes.bfloat16
    fp8 = ml_dtypes.float8_e4m3
    WmT = np.ascontiguousarray(W_mem.T).astype(fp8)
    def _pack(wt, FC=512, NP=128):
        # [D, F] -> [(F/FC)*P rows, (D/P)*FC cols]: row dc*P+p, col t*FC+f
        Dd, Ff = wt.shape
        kT, fC = Dd // NP, Ff // FC
        return np.ascontiguousarray(
            wt.reshape(kT, NP, fC, FC).transpose(2, 1, 0, 3)
            .reshape(fC * NP, kT * FC))
    WghT = _pack(np.ascontiguousarray(W_gate[:, :D].T)).astype(fp8)
    WgcT = _pack(np.ascontiguousarray(W_gate[:, D:].T)).astype(fp8)
    bg = np.ascontiguousarray(b_gate[None, :]).astype(bf16)
    nw = np.ascontiguousarray(norm_w[None, :], dtype=f32)
    ident = np.eye(P, dtype=f32).astype(bf16)
    ones = np.ones((1, P), dtype=bf16)

    in_maps = []
    for c in range(NC):
        b, h = c // H, c % H
        hs = hidden_states[b, h * BN:(h + 1) * BN, :]
        hsT = np.ascontiguousarray(hs.T)
        in_maps.append({
            "hiddenT8": _pack(hsT, FC=BN // 4).astype(fp8),
            "hidden": np.ascontiguousarray(hs, dtype=f32),
            "memT": np.ascontiguousarray(memory[b].T).astype(fp8),
            "WmT": WmT, "WghT": WghT, "WgcT": WgcT,
            "b_gate": bg, "norm_w": nw,
            "ident": ident, "ones": ones,
        })

    res = run_bass_kernel_spmd(prog, in_maps, core_ids=list(range(NC)))
    out = np.empty((B, N, D), dtype=f32)
    for c in range(NC):
        b, h = c // H, c % H
        out[b, h * BN:(h + 1) * BN, :] = res.results[c]["out"].astype(f32)
    return out

